# revision 13
# baseline (speedup 1.0000x reference)
"""Trainium2 Bass kernel for nn_GRIC_31550829756424 (GCN-attention block).

Data-parallel over batch: 8 batches -> 8 NeuronCores, one full batch per core.
Weights/B_bias replicated (B_bias pre-transposed + bf16-cast on host: weight
repack). All heavy matmuls run as float32r (1 cyc/row at N=512) except the
PV / output-projection matmuls which run bf16.

Self-contained: hardcodes all shapes; imports only the in-container concourse
stack.
"""

import sys

sys.path.insert(0, "/opt/trn_rl_repo")

import numpy as np
import ml_dtypes
from contextlib import ExitStack

import concourse.bass as bass
import concourse.tile as tile
from concourse import bacc
from concourse import mybir
from concourse.bass_utils import run_bass_kernel_spmd
from concourse.masks import make_identity

F32 = mybir.dt.float32
F32R = mybir.dt.float32r
BF16 = mybir.dt.bfloat16
AF = mybir.ActivationFunctionType
OP = mybir.AluOpType
AX = mybir.AxisListType

B = 8
N = 1024
D = 128
HEADS = 8
DV = 128
HD = HEADS * DV  # 1024
P = 128
NT = N // P  # 8 tiles of 128 rows
DK = 1.0 / float(np.sqrt(np.float32(D)))
EPS = 1e-5

_prog_cache = {}


def _ln_free(nc, small, out_ap, in_ap, eps_ap, gb, beb, extra_eps_ap=None):
    """LayerNorm over the free dim (width D) of [P, D] in_ap -> out_ap.

    If extra_eps_ap is given it is used as the bias of the Sqrt (must already
    be eps or eps*rs^2); otherwise eps_ap ([P,1] memset eps) is used.
    """
    s6 = small.tile([P, 6], F32, tag="s6")
    mv = small.tile([P, 2], F32, tag="mv")
    nc.vector.bn_stats(out=s6, in_=in_ap)
    nc.vector.bn_aggr(out=mv, in_=s6)
    std = small.tile([P, 1], F32, tag="std")
    nc.scalar.activation(
        out=std, in_=mv[:, 1:2], func=AF.Sqrt,
        bias=(extra_eps_ap if extra_eps_ap is not None else eps_ap),
    )
    rstd = small.tile([P, 1], F32, tag="rstd")
    nc.vector.reciprocal(out=rstd, in_=std)
    nc.vector.tensor_scalar(
        out=out_ap, in0=in_ap, scalar1=mv[:, 0:1], scalar2=rstd,
        op0=OP.subtract, op1=OP.mult,
    )
    nc.vector.tensor_mul(out=out_ap, in0=out_ap, in1=gb)
    nc.vector.tensor_add(out=out_ap, in0=out_ap, in1=beb)


def _bcast_load(nc, dst, src):
    """DMA-load 1D DRAM vector src [W] replicated across all P partitions of
    dst [P, W]."""
    rep = bass.AP(tensor=src.tensor, offset=src.offset, ap=[[0, P]] + list(src.ap))
    nc.gpsimd.dma_start(out=dst, in_=rep)


def _build_program():
    nc = bacc.Bacc(None)

    h_in = nc.declare_dram_parameter("h", [N, D], F32, isOutput=False)
    a_in = nc.declare_dram_parameter("a", [N, N], F32, isOutput=False)
    bt_in = nc.declare_dram_parameter("bt", [HEADS, N, N], BF16, isOutput=False)
    wq_in = nc.declare_dram_parameter("wq", [D, HD], F32, isOutput=False)
    wk_in = nc.declare_dram_parameter("wk", [D, HD], F32, isOutput=False)
    wv_in = nc.declare_dram_parameter("wv", [D, HD], F32, isOutput=False)
    bqr_in = nc.declare_dram_parameter("bqr", [P, NT], F32, isOutput=False)
    bkr_in = nc.declare_dram_parameter("bkr", [P, NT], F32, isOutput=False)
    bv_in = nc.declare_dram_parameter("bv", [HD], F32, isOutput=False)
    wo_in = nc.declare_dram_parameter("wo", [HD, D], BF16, isOutput=False)
    w1_in = nc.declare_dram_parameter("w1", [D, D], F32, isOutput=False)
    w2_in = nc.declare_dram_parameter("w2", [D, D], F32, isOutput=False)
    b1_in = nc.declare_dram_parameter("b1", [D, 1], F32, isOutput=False)
    b2_in = nc.declare_dram_parameter("b2", [D, 1], F32, isOutput=False)
    g_in = {}
    be_in = {}
    for i in range(4):
        g_in[i] = nc.declare_dram_parameter(f"g{i}", [D], F32, isOutput=False)
        be_in[i] = nc.declare_dram_parameter(f"be{i}", [D], F32, isOutput=False)
    out_dram = nc.declare_dram_parameter("out", [N, D], F32, isOutput=True)

    with tile.TileContext(nc) as tc, ExitStack() as ctx:
        consts = ctx.enter_context(tc.tile_pool(name="consts", bufs=1))
        persist = ctx.enter_context(tc.tile_pool(name="persist", bufs=1))
        small = ctx.enter_context(tc.tile_pool(name="small", bufs=12))
        stg = ctx.enter_context(tc.tile_pool(name="stg", bufs=3))
        psA = ctx.enter_context(
            tc.tile_pool(name="psA", bufs=2, space=bass.MemorySpace.PSUM))
        psT = ctx.enter_context(
            tc.tile_pool(name="psT", bufs=2, space=bass.MemorySpace.PSUM))
        psB = ctx.enter_context(
            tc.tile_pool(name="psB", bufs=4, space=bass.MemorySpace.PSUM))

        # ---- constants -------------------------------------------------
        ident = consts.tile([P, P], F32)
        make_identity(nc, ident)
        omi = consts.tile([P, P], F32)  # 1 - I
        nc.gpsimd.memset(omi, 1.0)
        nc.gpsimd.affine_select(
            out=omi, in_=omi, compare_op=OP.not_equal, fill=0.0,
            base=0, pattern=[[-1, P]], channel_multiplier=1)
        eps_t = consts.tile([P, 1], F32)
        nc.vector.memset(eps_t, EPS)

        gb = {}
        beb = {}
        for i in range(4):
            gb[i] = consts.tile([P, D], F32, name=f"g{i}b", tag=f"g{i}b")
            _bcast_load(nc, gb[i], g_in[i][:])
            beb[i] = consts.tile([P, D], F32, name=f"be{i}b", tag=f"be{i}b")
            _bcast_load(nc, beb[i], be_in[i][:])
        bvb = consts.tile([P, HD], F32)
        _bcast_load(nc, bvb, bv_in[:])

        w_sb = {}
        for nm, t in (("q", wq_in), ("k", wk_in), ("v", wv_in)):
            wstage = stg.tile([P, HD], F32, name=f"w{nm}s", tag="wstage")
            nc.gpsimd.dma_start(out=wstage, in_=t[:, :])
            w_sb[nm] = consts.tile([P, HD], F32R, name=f"w{nm}", tag=f"w{nm}")
            nc.vector.tensor_copy(out=w_sb[nm], in_=wstage)
        bqr = consts.tile([P, NT], F32)
        nc.gpsimd.dma_start(out=bqr, in_=bqr_in[:, :])
        bkr = consts.tile([P, NT], F32)
        nc.gpsimd.dma_start(out=bkr, in_=bkr_in[:, :])
        # W_O as [p, h, d] where partition p = f-within-tile, h = f-tile
        wo_sb = consts.tile([P, HEADS, D], BF16)
        nc.gpsimd.dma_start(
            out=wo_sb, in_=wo_in.rearrange("(hh p) d -> p hh d", p=P))
        w1_stage = stg.tile([P, D], F32, name="w1s", tag="wstage2")
        nc.gpsimd.dma_start(out=w1_stage, in_=w1_in[:, :])
        w1_sb = consts.tile([P, D], F32R, tag="w1")
        nc.vector.tensor_copy(out=w1_sb, in_=w1_stage)
        w2_stage = stg.tile([P, D], F32, name="w2s", tag="wstage2")
        nc.gpsimd.dma_start(out=w2_stage, in_=w2_in[:, :])
        w2_sb = consts.tile([P, D], F32R, tag="w2")
        nc.vector.tensor_copy(out=w2_sb, in_=w2_stage)
        b1_sb = consts.tile([P, 1], F32, tag="b1")
        nc.gpsimd.dma_start(out=b1_sb, in_=b1_in[:, :])
        b2_sb = consts.tile([P, 1], F32, tag="b2")
        nc.gpsimd.dma_start(out=b2_sb, in_=b2_in[:, :])

        # ---- phase A: H load + LN + transpose --------------------------
        h_sb = persist.tile([P, NT, D], F32, tag="h")
        nc.gpsimd.dma_start(out=h_sb, in_=h_in.rearrange("(t p) d -> p t d", p=P))
        hnT = persist.tile([P, N], F32R, tag="hnT")
        for i in range(NT):
            hn_t = stg.tile([P, D], F32, name="hn_t", tag="hn_t")
            _ln_free(nc, small, hn_t, h_sb[:, i, :], eps_t, gb[0], beb[0])
            pt = psT.tile([P, P], F32, tag="pt")
            nc.tensor.transpose(pt, hn_t, ident)
            nc.vector.tensor_copy(out=hnT[:, i * P:(i + 1) * P], in_=pt)

        # ---- persistent attention operands -----------------------------
        qT = persist.tile([P, HEADS, N], BF16, tag="qT")
        kT = persist.tile([P, HEADS, N], BF16, tag="kT")
        vna = persist.tile([P, NT, HEADS, DV + 1], BF16, tag="v")
        nc.vector.memset(vna[:, :, :, DV:DV + 1], 1.0)
        dis = persist.tile([P, NT], F32, tag="dis")

        # ---- phase B: adjacency prep -----------------------------------
        with tc.tile_pool(name="gcn", bufs=1) as gcn_pool:
            ahatTs = gcn_pool.tile([P, NT, N], F32R, tag="ahatTs")
            with tc.tile_pool(name="apool", bufs=2) as apool:
                for i in range(NT):
                    at = apool.tile([P, N], F32, tag="at")
                    nc.gpsimd.dma_start(out=at, in_=a_in[i * P:(i + 1) * P, :])
                    db = at[:, i * P:(i + 1) * P]
                    nc.vector.tensor_mul(out=db, in0=db, in1=omi)
                    nc.vector.tensor_add(out=db, in0=db, in1=ident)
                    rs = small.tile([P, 1], F32, tag="rs")
                    nc.vector.reduce_sum(out=rs, in_=at, axis=AX.X)
                    nc.vector.tensor_scalar_max(out=rs, in0=rs, scalar1=1.0)
                    sq = small.tile([P, 1], F32, tag="sq")
                    nc.scalar.activation(out=sq, in_=rs, func=AF.Sqrt)
                    di = dis[:, i:i + 1]
                    nc.vector.reciprocal(out=di, in_=sq)
                    nc.vector.tensor_scalar_mul(out=at, in0=at, scalar1=di)
                    for j in range(NT):
                        pt = psT.tile([P, P], F32, tag="pt")
                        nc.tensor.transpose(pt, at[:, j * P:(j + 1) * P], ident)
                        nc.vector.tensor_copy(
                            out=ahatTs[:, j, i * P:(i + 1) * P], in_=pt)

            # ---- phase C: GCN for Q, K, V ------------------------------
            with tc.tile_pool(name="xsp", bufs=1) as xsp:
                for w in ("q", "k", "v"):
                    xs = xsp.tile([P, NT, HD], F32R, tag="xs")
                    for i in range(NT):
                        for c in range(2):
                            ps = psA.tile([P, 512], F32, tag="ps")
                            nc.tensor.matmul(
                                ps,
                                hnT[:, i * P:(i + 1) * P],
                                w_sb[w][:, c * 512:(c + 1) * 512],
                                start=True, stop=True)
                            nc.vector.tensor_scalar_mul(
                                out=xs[:, i, c * 512:(c + 1) * 512], in0=ps,
                                scalar1=dis[:, i:i + 1])
                    if w in ("q", "k"):
                        dstT = qT if w == "q" else kT
                        bias = bqr if w == "q" else bkr
                        for hh in range(HEADS):
                            for c in range(2):
                                ps = psA.tile([P, 512], F32, tag="ps")
                                for j in range(NT):
                                    nc.tensor.matmul(
                                        ps,
                                        xs[:, j, hh * P:(hh + 1) * P],
                                        ahatTs[:, j, c * 512:(c + 1) * 512],
                                        start=(j == 0), stop=(j == NT - 1))
                                if w == "q":
                                    nc.vector.tensor_scalar(
                                        out=dstT[:, hh, c * 512:(c + 1) * 512],
                                        in0=ps, scalar1=bias[:, hh:hh + 1],
                                        scalar2=DK, op0=OP.add, op1=OP.mult)
                                else:
                                    nc.vector.tensor_scalar_add(
                                        out=dstT[:, hh, c * 512:(c + 1) * 512],
                                        in0=ps, scalar1=bias[:, hh:hh + 1])
                    else:
                        for i in range(NT):
                            for c in range(2):
                                ps = psA.tile([P, 512], F32, tag="ps")
                                for j in range(NT):
                                    nc.tensor.matmul(
                                        ps,
                                        ahatTs[:, j, i * P:(i + 1) * P],
                                        xs[:, j, c * 512:(c + 1) * 512],
                                        start=(j == 0), stop=(j == NT - 1))
                                nc.vector.tensor_add(
                                    out=vna[:, i, c * 4:(c + 1) * 4, 0:DV],
                                    in0=ps.rearrange("p (a b) -> p a b", a=4),
                                    in1=bvb[:, c * 512:(c + 1) * 512].rearrange(
                                        "p (a b) -> p a b", a=4))

        # ---- phase D: attention per head -------------------------------
        mhcT = persist.tile([P, HEADS, N], BF16, tag="mhcT")
        with tc.tile_pool(name="etp", bufs=2) as etp, \
             tc.tile_pool(name="btp", bufs=3) as btp:
            for hh in range(HEADS):
                et = etp.tile([P, NT, N], BF16, tag="et")
                for j in range(NT):
                    btt = btp.tile([P, N], BF16, tag="bt")
                    nc.gpsimd.dma_start(
                        out=btt, in_=bt_in[hh, j * P:(j + 1) * P, :])
                    for c in range(2):
                        ps = psA.tile([P, 512], F32, tag="ps")
                        nc.tensor.matmul(
                            ps,
                            kT[:, hh, j * P:(j + 1) * P],
                            qT[:, hh, c * 512:(c + 1) * 512],
                            start=True, stop=True)
                        st = stg.tile([P, 512], BF16, tag="st")
                        nc.vector.tensor_add(
                            out=st, in0=ps, in1=btt[:, c * 512:(c + 1) * 512])
                        nc.scalar.activation(
                            out=et[:, j, c * 512:(c + 1) * 512], in_=st,
                            func=AF.Exp)
                for i in range(NT):
                    pm = psB.tile([P, DV + 1], F32, tag="pm")
                    for j in range(NT):
                        nc.tensor.matmul(
                            pm,
                            et[:, j, i * P:(i + 1) * P],
                            vna[:, j, hh, :],
                            start=(j == 0), stop=(j == NT - 1))
                    # LN over d with the exact eps*rs^2 correction; the
                    # rowsum division cancels out of LayerNorm entirely.
                    s6 = small.tile([P, 6], F32, tag="s6")
                    mv = small.tile([P, 2], F32, tag="mv")
                    nc.vector.bn_stats(out=s6, in_=pm[:, 0:DV])
                    nc.vector.bn_aggr(out=mv, in_=s6)
                    rs_sb = small.tile([P, 1], F32, tag="rssb")
                    nc.vector.tensor_copy(out=rs_sb, in_=pm[:, DV:DV + 1])
                    t = small.tile([P, 1], F32, tag="t")
                    nc.vector.tensor_mul(out=t, in0=rs_sb, in1=rs_sb)
                    nc.scalar.mul(out=t, in_=t, mul=EPS)
                    std = small.tile([P, 1], F32, tag="std")
                    nc.scalar.activation(
                        out=std, in_=mv[:, 1:2], func=AF.Sqrt, bias=t)
                    rstd = small.tile([P, 1], F32, tag="rstd")
                    nc.vector.reciprocal(out=rstd, in_=std)
                    mh = stg.tile([P, DV], F32, tag="mh")
                    nc.vector.tensor_scalar(
                        out=mh, in0=pm[:, 0:DV], scalar1=mv[:, 0:1],
                        scalar2=rstd, op0=OP.subtract, op1=OP.mult)
                    nc.vector.tensor_mul(out=mh, in0=mh, in1=gb[1])
                    nc.vector.tensor_add(out=mh, in0=mh, in1=beb[1])
                    pt = psT.tile([P, P], F32, tag="pt")
                    nc.tensor.transpose(pt, mh, ident)
                    nc.vector.tensor_copy(
                        out=mhcT[:, hh, i * P:(i + 1) * P], in_=pt)

        # ---- phase E: output projection + MLP --------------------------
        o_ln = persist.tile([P, NT, D], F32, tag="oln")
        for i in range(NT):
            ps = psB.tile([P, DV + 1], F32, tag="pm")
            for hh in range(HEADS):
                nc.tensor.matmul(
                    ps[:, 0:D],
                    mhcT[:, hh, i * P:(i + 1) * P],
                    wo_sb[:, hh, :],
                    start=(hh == 0), stop=(hh == HEADS - 1))
            orow = stg.tile([P, D], F32, tag="orow")
            nc.vector.tensor_add(out=orow, in0=ps[:, 0:D], in1=h_sb[:, i, :])
            _ln_free(nc, small, o_ln[:, i, :], orow, eps_t, gb[2], beb[2])
        oT = persist.tile([P, N], F32R, tag="oT")
        for i in range(NT):
            pt = psT.tile([P, P], F32, tag="pt")
            nc.tensor.transpose(pt, o_ln[:, i, :], ident)
            nc.vector.tensor_copy(out=oT[:, i * P:(i + 1) * P], in_=pt)

        r1T = persist.tile([P, N], F32R, tag="r1T")
        for c in range(2):
            ps = psA.tile([P, 512], F32, tag="ps")
            nc.tensor.matmul(
                ps, w1_sb,
                oT[:, c * 512:(c + 1) * 512],
                start=True, stop=True)
            nc.scalar.activation(
                out=r1T[:, c * 512:(c + 1) * 512], in_=ps, func=AF.Relu,
                bias=b1_sb)
        r2T = persist.tile([P, N], F32, tag="r2T")
        for c in range(2):
            ps = psA.tile([P, 512], F32, tag="ps")
            nc.tensor.matmul(
                ps, w2_sb,
                r1T[:, c * 512:(c + 1) * 512],
                start=True, stop=True)
            nc.scalar.activation(
                out=r2T[:, c * 512:(c + 1) * 512], in_=ps, func=AF.Relu,
                bias=b2_sb)

        out_sb = persist.tile([P, NT, D], F32, tag="osb")
        for i in range(NT):
            pt = psT.tile([P, P], F32, tag="pt")
            nc.tensor.transpose(pt, r2T[:, i * P:(i + 1) * P], ident)
            r2 = stg.tile([P, D], F32, tag="r2")
            nc.vector.tensor_copy(out=r2, in_=pt)
            ro = stg.tile([P, D], F32, tag="ro")
            _ln_free(nc, small, ro, r2, eps_t, gb[3], beb[3])
            nc.vector.tensor_add(out=out_sb[:, i, :], in0=o_ln[:, i, :], in1=ro)
        nc.gpsimd.dma_start(
            out=out_dram.rearrange("(t p) d -> p t d", p=P), in_=out_sb)

    nc.compile()
    return nc


def _get_program():
    if "nc" not in _prog_cache:
        _prog_cache["nc"] = _build_program()
    return _prog_cache["nc"]


def kernel(**inputs):
    nc = _get_program()
    f32 = np.float32
    bf16 = ml_dtypes.bfloat16

    H = np.asarray(inputs["H"], dtype=f32)
    A = np.asarray(inputs["A"], dtype=f32)
    BT = np.ascontiguousarray(
        np.asarray(inputs["B_bias"], dtype=f32).transpose(0, 2, 1)).astype(bf16)
    base = {
        "bt": BT,
        "wq": np.asarray(inputs["W_Q"], dtype=f32),
        "wk": np.asarray(inputs["W_K"], dtype=f32),
        "wv": np.asarray(inputs["W_V"], dtype=f32),
        "bqr": np.ascontiguousarray(
            np.asarray(inputs["b_Q"], dtype=f32).reshape(NT, P).T),
        "bkr": np.ascontiguousarray(
            np.asarray(inputs["b_K"], dtype=f32).reshape(NT, P).T),
        "bv": np.asarray(inputs["b_V"], dtype=f32),
        "wo": np.asarray(inputs["W_O"], dtype=f32).astype(bf16),
        "w1": np.asarray(inputs["W1"], dtype=f32),
        "w2": np.asarray(inputs["W2"], dtype=f32),
        "b1": np.asarray(inputs["b1"], dtype=f32).reshape(D, 1),
        "b2": np.asarray(inputs["b2"], dtype=f32).reshape(D, 1),
    }
    for i, (g, be) in enumerate(
            (("g0", "be0"), ("g1", "be1"), ("g2", "be2"), ("g3", "be3"))):
        base[f"g{i}"] = np.asarray(inputs[g], dtype=f32)
        base[f"be{i}"] = np.asarray(inputs[be], dtype=f32)

    in_maps = []
    for c in range(B):
        m = dict(base)
        m["h"] = np.ascontiguousarray(H[c])
        m["a"] = np.ascontiguousarray(A[c])
        in_maps.append(m)

    res = run_bass_kernel_spmd(nc, in_maps, list(range(B)))
    out = np.stack([res.results[c]["out"] for c in range(B)], axis=0)
    return out.astype(np.float32)


if __name__ == "__main__":
    _get_program()
    print("program built ok")
